# revision 30
# baseline (speedup 1.0000x reference)
"""Trainium2 Bass kernel for nn_CrossAttention (3x3 scale-grid cross attention).

Reference computation (per batch b):
    WV_i = V_i @ W.T + b                    (video projection, i in 0..2)
    S_ij = (WV_i @ A_j.T) / sqrt(C)         [T, S] scores
    P_ij = softmax(S_ij, axis=-1)
    fv[i,j] = P_ij @ A_j        -> out[0, i, j, b]
    fa[j,i] = P_ij.T @ V_i      -> out[1, j, i, b]

Sharding: data-parallel over batch B=8 across the 8 NeuronCores (one batch
element per core). W/b replicated. Each core runs all 9 (i,j) attention pairs
for its batch element.

On-chip plan (per core): bf16 matmul paths with fp32 PSUM accumulation.
Softmax normalization is folded into the outputs (fv scaled at the PSUM->SBUF
copy, fa via a row-scaled copy of V). The host pre-transposes A/V/W (layout
prep only) so the device only transposes P. P^T transposes are software-
pipelined into the scores phase: the 8 transposed blocks of each P row-block
go to one PSUM bank and leave via one strided DVE copy.
"""

import numpy as np
from contextlib import ExitStack

import ml_dtypes

import concourse.bacc as bacc
import concourse.bass as bass
import concourse.mybir as mybir
import concourse.tile as tile
from concourse.bass_utils import run_bass_kernel_spmd
from concourse.masks import make_identity

BF16 = mybir.dt.bfloat16
F32 = mybir.dt.float32
AF = mybir.ActivationFunctionType

B, T, C = 8, 1024, 512
P = 128
NT = T // P   # 8 row blocks
ND = C // P   # 4 feature chunks
SCALE = 1.0 / float(np.sqrt(C))

_CACHE = {}

# Consecutive score matmuls share the stationary operand (fewer LDWEIGHTS)
# at the cost of alternating PSUM banks between back-to-back matmuls.
# Measured on HW: bank alternation costs ~2% — keep sequential.
_SCORES_BANK_INTERLEAVE = False


def _build(repeat=1, bench_internal=False, ps_t_bufs=1, ps_o_bufs=3,
           stage_bufs=4):
    key = ("nc", repeat, _SCORES_BANK_INTERLEAVE, bench_internal,
           ps_t_bufs, ps_o_bufs, stage_bufs)
    if key in _CACHE:
        return _CACHE[key]

    nc = bacc.Bacc("TRN2", target_bir_lowering=False, debug=False, num_devices=8)

    a_dram = [nc.dram_tensor(f"a{j}", [T, C], BF16, kind="ExternalInput").ap()
              for j in range(3)]
    v_dram = [nc.dram_tensor(f"v{i}", [T, C], BF16, kind="ExternalInput").ap()
              for i in range(3)]
    at_dram = [nc.dram_tensor(f"at{j}", [C, T], BF16, kind="ExternalInput").ap()
               for j in range(3)]
    vt_dram = [nc.dram_tensor(f"vt{i}", [C, T], BF16, kind="ExternalInput").ap()
               for i in range(3)]
    wt_dram = nc.dram_tensor("WT", [C, C], BF16, kind="ExternalInput").ap()
    b_dram = nc.dram_tensor("bvec", [ND, P, 1], F32, kind="ExternalInput").ap()
    out_kind = "Internal" if bench_internal else "ExternalOutput"
    out_dram = nc.dram_tensor("out", [2, 3, 3, T, C], F32, kind=out_kind).ap()
    sink_dram = (nc.dram_tensor("osink", [P, 1], F32, kind="ExternalOutput").ap()
                 if bench_internal else None)

    with ExitStack() as ctx:
        tc = ctx.enter_context(tile.TileContext(nc))

        const = ctx.enter_context(tc.tile_pool(name="const", bufs=1))
        big = ctx.enter_context(tc.tile_pool(name="big", bufs=1))
        work = ctx.enter_context(tc.tile_pool(name="work", bufs=1))
        small = ctx.enter_context(tc.tile_pool(name="small", bufs=1))
        stage = ctx.enter_context(tc.tile_pool(name="stage", bufs=stage_bufs))

        ps_s = ctx.enter_context(tc.tile_pool(name="ps_s", bufs=2, space="PSUM"))
        ps_t = ctx.enter_context(tc.tile_pool(name="ps_t", bufs=ps_t_bufs, space="PSUM"))
        ps_o = ctx.enter_context(tc.tile_pool(name="ps_o", bufs=ps_o_bufs, space="PSUM"))

        ident = const.tile([P, P], BF16, tag="ident", name="ident")
        make_identity(nc, ident[:])

        for _rep in range(repeat):
            _kernel_body(nc, tc, const, big, work, small, stage,
                         ps_s, ps_t, ps_o, ident,
                         a_dram, v_dram, at_dram, vt_dram, wt_dram, b_dram,
                         out_dram, sink_dram)

    nc.compile()
    _CACHE[key] = nc
    return nc


def _kernel_body(nc, tc, const, big, work, small, stage,
                 ps_s, ps_t, ps_o, ident,
                 a_dram, v_dram, at_dram, vt_dram, wt_dram, b_dram, out_dram,
                 sink_dram=None):
    # ---- load operands (bf16; transposed copies prepared on host) ----
    # Issue order = startup critical path: the first WV matmul needs all of
    # WT plus VT[0]; put those 8 tiles at the head of the DMA queues.
    WT = [big.tile([P, C], BF16, tag=f"WT{cc}", name=f"WT{cc}")
          for cc in range(ND)]
    VT = [[big.tile([P, T], BF16, tag=f"VT{i}_{cc}", name=f"VT{i}_{cc}")
           for cc in range(ND)] for i in range(3)]
    AT = [[big.tile([P, T], BF16, tag=f"AT{j}_{cc}", name=f"AT{j}_{cc}")
           for cc in range(ND)] for j in range(3)]
    # interleave (WT[cc], VT[0][cc]) pairs: matches the first WV block's
    # consumption order so the PE starts ~1us sooner
    for cc in range(ND):
        nc.sync.dma_start(WT[cc][:], wt_dram[cc * P:(cc + 1) * P, :])
        nc.sync.dma_start(VT[0][cc][:], vt_dram[0][cc * P:(cc + 1) * P, :])

    b_sb = []
    for dc in range(ND):
        t_ = const.tile([P, 1], F32, tag=f"b{dc}", name=f"b{dc}")
        nc.sync.dma_start(t_[:], b_dram[dc])
        b_sb.append(t_)

    for i in range(1, 3):
        for cc in range(ND):
            nc.sync.dma_start(VT[i][cc][:], vt_dram[i][cc * P:(cc + 1) * P, :])
    for cc in range(ND):
        nc.sync.dma_start(AT[0][cc][:], at_dram[0][cc * P:(cc + 1) * P, :])

    Abf = [[big.tile([P, C], BF16, tag=f"A{j}_{tb}", name=f"A{j}_{tb}")
            for tb in range(NT)] for j in range(3)]
    Vbf = [[big.tile([P, C], BF16, tag=f"V{i}_{tb}", name=f"V{i}_{tb}")
            for tb in range(NT)] for i in range(3)]
    for tb in range(NT):
        nc.sync.dma_start(Vbf[0][tb][:], v_dram[0][tb * P:(tb + 1) * P, :])
    for tb in range(NT):
        nc.sync.dma_start(Abf[0][tb][:], a_dram[0][tb * P:(tb + 1) * P, :])

    def load_bulk_inputs():
        # inputs for pairs after (0,0), issued once pair (0,0) is emitted so
        # its XBAR transposes aren't stuck behind 6MB of loads in the SP
        # queue (consumers wait on the tile semaphores either way)
        for j in range(1, 3):
            for cc in range(ND):
                nc.sync.dma_start(AT[j][cc][:],
                                  at_dram[j][cc * P:(cc + 1) * P, :])
        for j in range(1, 3):
            for tb in range(NT):
                nc.gpsimd.dma_start(Abf[j][tb][:],
                                    a_dram[j][tb * P:(tb + 1) * P, :])
        for i in range(1, 3):
            for tb in range(NT):
                nc.gpsimd.dma_start(Vbf[i][tb][:],
                                    v_dram[i][tb * P:(tb + 1) * P, :])
    load_bulk_inputs()

    # ---- WV^T_i[d, t] = W^T @ V^T_i + b (bf16 out, bias folded in) ----
    # cc outer / th inner: consecutive matmuls share the stationary operand,
    # halving LDWEIGHTS traffic (the two halves accumulate in two banks).
    # Only WV[0] is emitted up front (startup-critical bytes = WT+VT[0]);
    # WV[1]/WV[2] are emitted between the early pairs, long before use.
    WVT = [[big.tile([P, T], BF16, tag=f"WVT{i}_{dc}", name=f"WVT{i}_{dc}")
            for dc in range(ND)] for i in range(3)]

    def emit_wv(i):
        for dc in range(ND):
            po2 = [ps_o.tile([P, C], F32, tag="o", name="o") for _ in range(2)]
            for cc in range(ND):
                for th in range(2):
                    nc.tensor.matmul(po2[th][:], WT[cc][:, dc * P:(dc + 1) * P],
                                     VT[i][cc][:, th * C:(th + 1) * C],
                                     start=(cc == 0), stop=(cc == ND - 1))
            for th in range(2):
                nc.scalar.activation(WVT[i][dc][:, th * C:(th + 1) * C],
                                     po2[th][:], AF.Identity,
                                     bias=b_sb[dc][:], scale=1.0)

    for _i in range(3):
        emit_wv(_i)

    # ---- main loop over the 9 attention pairs ----
    for i in range(3):
        for j in range(3):
            Pt = [work.tile([P, T], BF16, tag=f"P{tb}", name=f"P{tb}")
                  for tb in range(NT)]
            # PTa layout [p, tb, sc, t]: each tb's transpose destination is a
            # contiguous [sc, t] region (dma_start_transpose requires a
            # contiguous SBUF dst; folded rows land as s = sc*128 + p).
            PTa = work.tile([P, NT * T], BF16, tag="PTall", name="PTall")
            PTv = PTa.rearrange("p (tb sc t) -> p tb sc t", tb=NT, sc=NT)
            recip = [small.tile([P, 1], F32, tag=f"rc{tb}", name=f"rc{tb}")
                     for tb in range(NT)]
            Vr = [work.tile([P, C], BF16, tag=f"Vr{tb}", name=f"Vr{tb}")
                  for tb in range(NT)]

            def transpose_block(tb):
                # P[tb] [128t, 1024s] -> P^T via one XBAR transpose DMA per
                # block; the late-needed Abf/Vbf bulk loads ride the idle
                # GPSIMD SWDGE queue so even pair 0's transposes see a
                # near-empty SP queue.
                nc.sync.dma_start_transpose(PTv[:, tb], Pt[tb][:])

            for tb in range(NT):
                # one [128, 1024] score block = 2 PSUM banks; each matmul
                # stays within one bank. dc outer / h inner shares the
                # stationary operand between consecutive matmuls.
                ps = ps_s.tile([P, T], F32, tag="s", name="s")
                if _SCORES_BANK_INTERLEAVE:
                    for dc in range(ND):
                        for h in range(2):
                            nc.tensor.matmul(ps[:, h * C:(h + 1) * C],
                                             WVT[i][dc][:, tb * P:(tb + 1) * P],
                                             AT[j][dc][:, h * C:(h + 1) * C],
                                             start=(dc == 0), stop=(dc == ND - 1))
                else:
                    for h in range(2):
                        for dc in range(ND):
                            nc.tensor.matmul(ps[:, h * C:(h + 1) * C],
                                             WVT[i][dc][:, tb * P:(tb + 1) * P],
                                             AT[j][dc][:, h * C:(h + 1) * C],
                                             start=(dc == 0), stop=(dc == ND - 1))
                rsum = small.tile([P, 1], F32, tag=f"rsum{tb}", name=f"rsum{tb}")
                nc.scalar.activation(Pt[tb][:], ps[:], AF.Exp, scale=SCALE,
                                     accum_out=rsum[:])
                nc.vector.reciprocal(recip[tb][:], rsum[:])
                nc.vector.tensor_scalar_mul(Vr[tb][:], Vbf[i][tb][:],
                                            recip[tb][:])
                if tb >= 1:
                    transpose_block(tb - 1)
            transpose_block(NT - 1)

            # fa[j,i] = P_raw^T @ (diag(recip) @ V_i)
            # fv[i,j] = diag(recip) @ (P_raw @ A_j)
            # interleaved so the kernel tail drains two engines in parallel
            for k in range(NT):
                po = ps_o.tile([P, C], F32, tag="o", name="o")
                for tb in range(NT):
                    nc.tensor.matmul(po[:], Pt[tb][:, k * P:(k + 1) * P],
                                     Vr[tb][:],
                                     start=(tb == 0), stop=(tb == NT - 1))
                st = stage.tile([P, C], F32, tag="st", name="st")
                nc.vector.tensor_copy(st[:], po[:])
                # last pair: spread outputs over both queues to shorten the
                # end-of-kernel drain (SP is transpose-only and idle by then)
                oeng = nc.sync if (i == 2 and j == 2 and k >= 4) else nc.scalar
                oeng.dma_start(out_dram[1, j, i, k * P:(k + 1) * P, :],
                               st[:])

                po = ps_o.tile([P, C], F32, tag="o", name="o")
                for sc in range(NT):
                    nc.tensor.matmul(
                        po[:], PTv[:, k, sc],
                        Abf[j][sc][:],
                        start=(sc == 0), stop=(sc == NT - 1))
                st = stage.tile([P, C], F32, tag="st", name="st")
                nc.scalar.activation(st[:], po[:], AF.Copy, bias=0.0,
                                     scale=recip[k][:])
                oeng.dma_start(out_dram[0, i, j, k * P:(k + 1) * P, :],
                               st[:])
    if sink_dram is not None:
        nc.sync.dma_start(sink_dram, b_sb[0][:])


def _prep_in_maps(a0, a1, a2, v0, v1, v2, W, b):
    bf = ml_dtypes.bfloat16
    a_bf = [np.asarray(x, dtype=np.float32).astype(bf) for x in (a0, a1, a2)]
    v_bf = [np.asarray(x, dtype=np.float32).astype(bf) for x in (v0, v1, v2)]
    wt_bf = np.ascontiguousarray(np.asarray(W, dtype=np.float32).astype(bf).T)
    b_r = np.ascontiguousarray(
        np.asarray(b, dtype=np.float32).reshape(ND, P, 1))
    in_maps = []
    for bi in range(B):
        m = {f"a{j}": np.ascontiguousarray(a_bf[j][bi]) for j in range(3)}
        m.update({f"v{i}": np.ascontiguousarray(v_bf[i][bi]) for i in range(3)})
        m.update({f"at{j}": np.ascontiguousarray(a_bf[j][bi].T)
                  for j in range(3)})
        m.update({f"vt{i}": np.ascontiguousarray(v_bf[i][bi].T)
                  for i in range(3)})
        m["WT"] = wt_bf
        m["bvec"] = b_r
        in_maps.append(m)
    return in_maps


def run(inputs, trace=False, tmpdir=None):
    """Build+run on 8 cores; returns (full_output, BassKernelResults)."""
    nc = _build()
    in_maps = _prep_in_maps(**inputs)
    res = run_bass_kernel_spmd(nc, in_maps, list(range(B)), trace=trace,
                               tmpdir=tmpdir)
    out = np.empty((2, 3, 3, B, T, C), dtype=np.float32)
    for bi in range(B):
        out[:, :, :, bi] = res.results[bi]["out"]
    return out, res


def kernel(a0, a1, a2, v0, v1, v2, W, b):
    out, _ = run(dict(a0=a0, a1=a1, a2=a2, v0=v0, v1=v1, v2=v2, W=W, b=b))
    return out

